# revision 71
# baseline (speedup 1.0000x reference)
"""BCQLinear packed forward on 8 Trainium2 NeuronCores.

Column-parallel sharding: binary/alpha/bias sharded along out_features
(4096 -> 8 x 512); activations replicated; host concatenates o-shards.

Per-core pipeline (fp8 DoubleRow formulation):
  W[o,i] = sum_b alpha[o,g,b] * B_b[o,i]   (i = 128 g + a)
  out    = x @ W^T + bias

  1. DVE/GPSIMD scale the +-1 bit-planes by alpha into bf16 planes
     (o on partitions), streamed per g-pair kk = (g0,g1).
  2. PE transpose-accumulates the 3 planes per [128,128] tile into a
     bf16 PSUM tile -> true W^T chunk [a, o].
  3. ACT casts the chunk to fp8e4 (wt_hi); DVE/GPSIMD write the
     residual to fp8e5 (wt_lo = W^T - wt_hi).  x is split on the host:
     x_hi = e4(x), x_lo = e5(x - x_hi).
  4. PE runs DoubleRow fp8 matmuls (contract 2 k-tiles/instruction):
     3 legs per (m, kk): x_hi*W_hi + x_lo*W_hi + x_hi*W_lo.
     A window of m-blocks accumulates in PSUM while dequant streams
     (wavefront); remaining m-blocks run as a pure-matmul tail.
  5. DVE adds bias, casts to bf16, DMA out.

Shapes hardcoded for this instance:
  input [2,1024,4096] f32; binary [4096,32,128,3] f32 (+-1);
  alpha [4096,32,3] f32; bias [4096] f32.
"""

import numpy as np
from contextlib import ExitStack

import bass_rust
import concourse.bass as bass
import concourse.mybir as mybir
import concourse.tile as tile
from concourse.bass_utils import run_bass_kernel_spmd
from concourse.masks import make_identity


def _legalize_waits(nc, max_waits=1):
    """Walrus allows only one sync-wait on (at least) DVE TensorTensor
    instructions. Move excess waits onto injected same-engine NoOps placed
    immediately before the instruction."""
    seq = 0
    for fn in nc.m.functions:
        for blk in fn.blocks:
            new_insts = []
            changed = False
            for inst in blk.instructions:
                si = inst.sync_info
                if si is not None and len(si.on_wait) > max_waits:
                    waits = list(si.on_wait)
                    for w in waits[:-max_waits]:
                        nop = mybir.InstNoOp(name=f"wlegal-{seq}")
                        seq += 1
                        nop.engine = inst.engine
                        nop.sync_info = bass_rust.SyncInfo(
                            on_wait=[w], on_update=[])
                        new_insts.append(nop)
                    inst.sync_info = bass_rust.SyncInfo(
                        on_wait=waits[-max_waits:],
                        on_update=list(si.on_update))
                    changed = True
                new_insts.append(inst)
            if changed:
                blk.instructions = new_insts


P = 128          # partitions
N_CORES = 8
B, S = 2, 1024
MS = B * S       # 2048 tokens
I = 4096         # in_features
O = 4096         # out_features
O_SH = O // N_CORES  # 512 per core
G, A, NB = 32, 128, 3
KK = G // 2      # 16 g-pairs (DoubleRow contracts 2 k-tiles)
MB = MS // P     # 16 token blocks
OT = O_SH // P   # 4 o-tiles per core

F32 = mybir.dt.float32
F32R = mybir.dt.float32r
BF16 = mybir.dt.bfloat16
FP16 = mybir.dt.float16
E4 = mybir.dt.float8e4
E5 = mybir.dt.float8e5
DR = mybir.MatmulPerfMode.DoubleRow

_CACHED = {}
PE_LOG = []  # (label, ...) per PE matmul in emission order, for profiling


def build_nc(window: int = 6, admit_off: int = 2, admit_num: int = 17,
             admit_den: int = 20, x_bufs: int = 15, bq_bufs: int = 4,
             plane_bufs: int = 9, plane_dt: str = "bf16", bq_step: int = 1,
             wlo_dt: str = "e4", xlo_dt: str = "e4", tr_bufs: int = 2, xw_split: bool = False,
             out_bufs: int = 2, x2_start: int = 6, x2_queue: str = "sp", x2_mark: bool = False, scale_alt: bool = False, sub_split: int = 0, scale_tsplit_bit: int = -1, s0_tsplit: int = 0, ramp_flip: int = 0, sub_sched: str = "", spill_mode: bool = True,
             spill1: int = 12, reload1: int = 15, burst: int = 3, prt_bufs: int = 8,
             spill_cap: int = 10, spill_cap2: int = 0,
             wlo_via: str = "dve", ps_direct: bool = False, tail_split: bool = False,
             scale_gps_bits=(2,), sub_gps_ts=(),
             preacc: int = 0, pa_m1_eng: str = "gps", pa_tmp_bufs: int = 3,
             pa_eng: str = "dgd", fin_gps: bool = False, pa_from_s: int = 0,
             bq0_split: bool = False, lo_defer: bool = False,
             al_split: int = 0, pe_warmup: int = 0,
             warm_ap: int = 128, bq_head: int = 1,
             bq_q: str = "act", out_q_last: str = "sp",
             head_merge: bool = False, sub_gps_from: int = 99,
             act_from: int = 99, act_until: int = 99, act_b: int = 0,
             act_t: int = 1, pa_act: bool = False) -> bass.Bass:
    PE_LOG.clear()
    nc = bass.Bass("TRN2", target_bir_lowering=False, debug=False)
    PDT = {"bf16": BF16, "f32r": F32R, "fp16": FP16}[plane_dt]
    WLO = {"e5": E5, "e4": E4}[wlo_dt]
    XLO = {"e5": E5, "e4": E4}[xlo_dt]

    # Host-staged layouts (pure relayouts/casts of the sharded inputs):
    #  xhi/xlo [MB, P, KK, 2, P]: x[m*128+j, (2kk+t)*128+p] fp8 hi/lo split
    #  bq  [KK, P, OT, NB, 2, A]: binary[ot*128+p(o), g=2kk+t, a, b]
    #  al  [P, OT, G, NB]       : alpha[ot*128+p, g, b]
    #  biasr [P, O_SH]          : bias replicated across partitions
    U8 = mybir.dt.uint8
    HEADB = OT * G * NB * 2 + OT * NB * 2 * A  # fp16 alpha + e4 bq chunk 0
    xhi_d = nc.dram_tensor("xhi", [MB, P, KK, 2, P], E4, kind="ExternalInput").ap()
    xlo_d = nc.dram_tensor("xlo", [MB, P, KK, 2, P], XLO, kind="ExternalInput").ap()
    # bq grouped in chunks of bq_step g-pairs per DMA
    bq_d = nc.dram_tensor("bq", [KK, P, OT, NB, 2, A], E4, kind="ExternalInput").ap()
    act_on = act_from < 99
    if head_merge:
        # fp16 alpha and the first bq chunk packed into one DMA: a single
        # semaphore/queue slot so the first scales fire as early as possible
        head_d = nc.dram_tensor("head", [P, HEADB], U8,
                                kind="ExternalInput").ap()
    if not head_merge or act_on:
        # walrus' lower_act requires f32 scale APs, so the ACT offload reads
        # a separate f32 alpha even when the head carries the fp16 copy
        al_d = nc.dram_tensor("al", [P, OT, G, NB], F32, kind="ExternalInput").ap()
    bias_d = nc.dram_tensor("biasr", [P, O_SH], F32, kind="ExternalInput").ap()
    out_d = nc.dram_tensor("out", [MS, O_SH], BF16, kind="ExternalOutput").ap()
    out_t = out_d.rearrange("(mb p) o -> mb p o", p=P)
    if ps_direct:
        # last m-block bypasses the SBUF hop: bias preloaded into its PSUM
        # bank, accumulator DMAd straight to DRAM in f32. (Unused: bass
        # dma_start cannot source PSUM; kept for reference.)
        o15_d = nc.dram_tensor("o15", [P, O_SH], F32, kind="ExternalOutput").ap()

    mult = mybir.AluOpType.mult
    add = mybir.AluOpType.add
    sub = mybir.AluOpType.subtract

    # static admission schedule for the m-block wavefront
    admits = [[] for _ in range(KK)]
    adm = 0
    for s in range(KK):
        want = min(window, admit_off + (admit_num * s) // admit_den)
        while adm < want:
            admits[s].append(adm)
            adm += 1
    while adm < window:
        admits[KK - 1].append(adm)
        adm += 1

    with tile.TileContext(nc) as tc, ExitStack() as ctx:
        const = ctx.enter_context(tc.tile_pool(name="const", bufs=1))
        wtp = ctx.enter_context(tc.tile_pool(name="wt", bufs=1))
        bqp = ctx.enter_context(tc.tile_pool(name="bq", bufs=bq_bufs))
        plp = ctx.enter_context(tc.tile_pool(name="pl", bufs=plane_bufs))
        xph = ctx.enter_context(tc.tile_pool(name="xh", bufs=x_bufs))
        xpl = ctx.enter_context(tc.tile_pool(name="xl", bufs=x_bufs))
        outp = ctx.enter_context(tc.tile_pool(name="out", bufs=out_bufs))
        prtp = ctx.enter_context(tc.tile_pool(name="prt", bufs=prt_bufs))
        tmpp = (ctx.enter_context(tc.tile_pool(name="pat", bufs=pa_tmp_bufs))
                if preacc else None)
        ps_mm = ctx.enter_context(tc.tile_pool(name="psmm", bufs=window, space="PSUM"))
        ps_tr = ctx.enter_context(tc.tile_pool(name="pstr", bufs=tr_bufs, space="PSUM"))

        if pe_warmup:
            # dependency-free dummy matmuls: keep PE busy from ~0.6us so the
            # p-state ramp (3us of 2x-slow cycles) burns during the DMA wait
            # instead of on the first real transposes/legs
            dum = const.tile([P, P], BF16, tag="dum")
            nc.gpsimd.memset(dum[:, :warm_ap], 0)
            wps = ps_tr.tile([P, OT * P], F32, name="warm", tag="trt")
            for i in range(pe_warmup):
                PE_LOG.append(("warm", i))
                nc.tensor.matmul(wps[:, :warm_ap], dum[:, :warm_ap],
                                 dum[:, :warm_ap], start=True, stop=True)

        ident_f = const.tile([P, P], F32)
        make_identity(nc, ident_f)
        ident = const.tile([P, P], BF16, tag="identp")
        nc.vector.tensor_copy(ident, ident_f)
        if wlo_via == "pe":
            nident = const.tile([P, P], BF16, tag="identn")
            nc.vector.tensor_scalar(nident, ident_f, -1.0, None, mult)
        ALB = OT * G * NB * 2
        al32_sb = None
        if head_merge:
            assert bq_step == 1
            al_split = 0
            bq0_split = False
            head_sb = const.tile([P, HEADB], mybir.dt.uint8, tag="head")
            al_sb = head_sb[:, :ALB].bitcast(FP16).rearrange(
                "p (ot g nb) -> p ot g nb", ot=OT, g=G)
            bq0_view = head_sb[:, ALB:].bitcast(E4).rearrange(
                "p (ot nb t a) -> p ot nb t a", ot=OT, nb=NB, t=2)
            if act_on:
                al32_sb = const.tile([P, OT, G, NB], F32, tag="al32")
        else:
            al_sb = const.tile([P, OT, G, NB], F32, tag="al")
            al32_sb = al_sb
        bias_sb = const.tile([P, O_SH], F32, tag="bias")
        if head_merge:
            nc.sync.dma_start(head_sb, head_d)
            if act_on:
                # needed from step act_from (~15us in) -- ride the ACT queue
                nc.scalar.dma_start(al32_sb, al_d)
        elif al_split:
            nc.sync.dma_start(al_sb[:, :, :al_split], al_d[:, :, :al_split])
            nc.sync.dma_start(al_sb[:, :, al_split:], al_d[:, :, al_split:])
        else:
            nc.sync.dma_start(al_sb, al_d)

        wt_hi = wtp.tile([P, G, O_SH], E4, tag="hi")
        wt_lo = wtp.tile([P, G, O_SH], WLO, tag="lo")

        # ---- all input DMAs, interleaved so x m-blocks arrive while the
        # binary g-pair chunks stream
        bq_sb, xhi_sb, xlo_sb = {}, {}, {}

        # bq/al stream on the ACT DGE queue so the small per-step binary
        # chunks are not head-of-line blocked behind the big x transfers
        # on the SP queue
        bq_dc = bq_d.rearrange("(c g) p ot nb t a -> c p g ot nb t a", g=bq_step)

        def load_bq(c):
            if c == 0 and head_merge:
                bq_sb[0] = bq0_view
                return
            tile_ = bqp.tile([P, bq_step, OT, NB, 2, A], E4, name="bqt")
            for g in range(bq_step):
                bq_sb[c * bq_step + g] = tile_[:, g]
            # head chunks ride the lower-latency SP queue (startup); with
            # bq_q="sp" all chunks do, keeping them ahead of the x stream
            # in the serialized DMA device order
            q = (nc.sync if (c < bq_head or bq_q == "sp")
                 else nc.scalar)
            if c == 0 and bq0_split:
                # per-bit-plane DMAs so the first scale starts ~3x sooner
                for b in range(NB):
                    q.dma_start(tile_[:, :, :, b], bq_dc[c][:, :, :, b])
            else:
                q.dma_start(tile_, bq_dc[c])

        # DMA order: bq chunks interleaved with only the window m-block x
        # loads (phase-1 critical), then the remaining bq burst, then the
        # phase-2 x stream. bq rides the ACT DGE queue, x the SP queue.
        def load_x(m):
            xhi_sb[m] = xph.tile([P, KK, 2, P], E4, name="xht")
            nc.sync.dma_start(xhi_sb[m], xhi_d[m])
            xlo_sb[m] = xpl.tile([P, KK, 2, P], XLO, name="xlt")
            nc.sync.dma_start(xlo_sb[m], xlo_d[m])

        NC = KK // bq_step
        nbq = min(3, NC)
        for c in range(nbq):
            load_bq(c)
        if xw_split:
            KH = KK // 2
            for m in range(window):
                xhi_sb[m] = xph.tile([P, KK, 2, P], E4, name="xht")
                xlo_sb[m] = xpl.tile([P, KK, 2, P], XLO, name="xlt")
                nc.sync.dma_start(xhi_sb[m][:, :KH], xhi_d[m][:, :KH])
                nc.sync.dma_start(xlo_sb[m][:, :KH], xlo_d[m][:, :KH])
                if nbq < NC:
                    load_bq(nbq)
                    nbq += 1
            while nbq < NC:
                load_bq(nbq)
                nbq += 1
            for m in range(window):
                nc.sync.dma_start(xhi_sb[m][:, KH:], xhi_d[m][:, KH:])
                nc.sync.dma_start(xlo_sb[m][:, KH:], xlo_d[m][:, KH:])
        else:
            for m in range(window):
                load_x(m)
                if nbq < NC:
                    load_bq(nbq)
                    nbq += 1
            while nbq < NC:
                load_bq(nbq)
                nbq += 1
        nc.scalar.dma_start(bias_sb, bias_d)
        # phase-2 x loads are issued from the DVE DGE queue inside the
        # step loop (paced by compute) so they do not crowd the DMA
        # device while the phase-1-critical bq/x-window transfers stream
        xq2 = list(range(window, MB))
        for m in xq2:
            xhi_sb[m] = xph.tile([P, KK, 2, P], E4, name="xht")
            xlo_sb[m] = xpl.tile([P, KK, 2, P], XLO, name="xlt")
            if x2_queue in ("act", "sp"):
                q = nc.scalar if x2_queue == "act" else nc.sync
                if x2_mark:
                    nc.gpsimd.memset(xhi_sb[m][:, 0, 0, :4], 0)
                    nc.gpsimd.memset(xlo_sb[m][:, 0, 0, :4], 0)
                q.dma_start(xhi_sb[m], xhi_d[m])
                q.dma_start(xlo_sb[m], xlo_d[m])

        # ---- per-step state
        part_sb = {}  # m -> spilled partial (bf16)
        planes = {}   # (s, b) -> bf16 plane tile [P, OT, 2, A]
        trs = {}      # (s, t) -> psum tile
        ps_of = {}    # m -> psum accumulator
        next_kk = {}  # m -> next kk to emit legs for
        nleg = {m: 0 for m in range(MB)}
        out_done = set()

        def scale(s, b):
            gbits = scale_gps_bits if (not scale_alt or s % 2 == 0) else (1, 2)
            pl = plp.tile([P, OT, 2, A], PDT, name="plt")
            planes[(s, b)] = pl
            eng = nc.gpsimd if b in gbits else nc.vector
            if b == act_b and act_from <= s < act_until:
                # offload one t-half to ACT as per-ot per-partition-scale
                # muls (Identity, the lowering-supported scale-AP form);
                # the other half stays a single broadcast TT
                for ot in range(OT):
                    nc.scalar.mul(pl[:, ot, act_t],
                                  bq_sb[s][:, ot, b, act_t],
                                  al32_sb[:, ot, 2 * s + act_t, b:b + 1])
                to = 1 - act_t
                eng.tensor_tensor(
                    pl[:, :, to], bq_sb[s][:, :, b, to],
                    al_sb[:, :, 2 * s + to:2 * s + to + 1, b:b + 1]
                    .to_broadcast([P, OT, 1, A])[:, :, 0],
                    mult)
                return
            eng.tensor_tensor(
                pl, bq_sb[s][:, :, b],
                al_sb[:, :, 2 * s:2 * s + 2, b:b + 1].to_broadcast([P, OT, 2, A]),
                mult)

        def scale_preacc(s):
            # fused dequant: bit-planes multiplied at 1x (fp8 input) but
            # pre-accumulated with 2x all-16-bit adds, so PE transposes one
            # plane (preacc=2) or two (preacc=1) instead of three
            def albc(b):
                return al_sb[:, :, 2 * s:2 * s + 2, b:b + 1].to_broadcast(
                    [P, OT, 2, A])
            E = {"d": nc.vector, "g": nc.gpsimd}
            m0, m1, m2 = (E[c] for c in pa_eng)
            tmp1 = tmpp.tile([P, OT, 2, A], PDT, name="pat1")
            m1.tensor_tensor(tmp1, bq_sb[s][:, :, 1], albc(1), mult)
            acc = plp.tile([P, OT, 2, A], PDT, name="plt")
            planes[(s, 0)] = acc
            if pa_act == 1:
                # ACT takes one t-half of the b0 mult (per-ot scale muls)
                for ot in range(OT):
                    nc.scalar.mul(acc[:, ot, act_t],
                                  bq_sb[s][:, ot, 0, act_t],
                                  al32_sb[:, ot, 2 * s + act_t, 0:1])
                to = 1 - act_t
                m0.tensor_tensor(
                    acc[:, :, to], bq_sb[s][:, :, 0, to],
                    al_sb[:, :, 2 * s + to:2 * s + to + 1, 0:1]
                    .to_broadcast([P, OT, 1, A])[:, :, 0], mult)
            else:
                m0.tensor_tensor(acc, bq_sb[s][:, :, 0], albc(0), mult)
            if preacc == 2:
                tmp2 = tmpp.tile([P, OT, 2, A], PDT, name="pat2")
                if pa_act == 2:
                    # ACT takes one t-half of the b2 mult (DVE relief in the
                    # vector-bound preacc region)
                    for ot in range(OT):
                        nc.scalar.mul(tmp2[:, ot, act_t],
                                      bq_sb[s][:, ot, 2, act_t],
                                      al32_sb[:, ot, 2 * s + act_t, 2:3])
                    to = 1 - act_t
                    m2.tensor_tensor(
                        tmp2[:, :, to], bq_sb[s][:, :, 2, to],
                        al_sb[:, :, 2 * s + to:2 * s + to + 1, 2:3]
                        .to_broadcast([P, OT, 1, A])[:, :, 0], mult)
                else:
                    m2.tensor_tensor(tmp2, bq_sb[s][:, :, 2], albc(2), mult)
                nc.vector.tensor_tensor(acc, acc, tmp1, add)
                nc.vector.tensor_tensor(acc, acc, tmp2, add)
            else:
                p2 = plp.tile([P, OT, 2, A], PDT, name="plt2")
                planes[(s, 1)] = p2
                m2.tensor_tensor(p2, bq_sb[s][:, :, 2], albc(2), mult)
                nc.vector.tensor_tensor(acc, acc, tmp1, add)

        def transposes_preacc(s):
            nb_pl = 1 if preacc == 2 else 2
            for t in range(2):
                ps = ps_tr.tile([P, OT * P], F32, name="trt")
                trs[(s, t)] = ps
                for ot in range(OT):
                    for b in range(nb_pl):
                        PE_LOG.append(("tr", s, t, ot, b))
                        nc.tensor.matmul(
                            ps[:, ot * P:(ot + 1) * P],
                            planes[(s, b)][:, ot, t], ident,
                            start=(b == 0), stop=(b == nb_pl - 1))

        def transposes(s):
            # transpose-accumulate via a REGULAR matmul against a constant
            # identity (out[a,o] = sum_o' plane[o',a] I[o',o]): same cost
            # as is_transpose (1.0 cyc/row keyed on the moving identity)
            # but uses the standard f32 PSUM accumulation path, which is
            # what real HW supports for multi-plane accumulation.
            for t in range(2):
                ps = ps_tr.tile([P, OT * P], F32, name="trt")
                trs[(s, t)] = ps
                for ot in range(OT):
                    for b in range(NB):
                        PE_LOG.append(("tr", s, t, ot, b))
                        nc.tensor.matmul(
                            ps[:, ot * P:(ot + 1) * P],
                            planes[(s, b)][:, ot, t], ident,
                            start=(ot == 0 and b == 0),
                            stop=(ot == OT - 1 and b == NB - 1))

        def cast_hi(s, t):
            nc.scalar.copy(wt_hi[:, 2 * s + t, :], trs[(s, t)])

        def neg_mm(s, t):
            # accumulate -W_hi into the W PSUM chunk (after the hi cast has
            # read it): the chunk becomes W_lo in place, freeing DVE/GPSIMD
            # from the residual subtract
            PE_LOG.append(("neg", s, t))
            nc.tensor.matmul(trs[(s, t)], nident, wt_hi[:, 2 * s + t, :],
                             start=False, stop=True, skip_group_check=True)

        def cast_lo(s, t):
            nc.scalar.copy(wt_lo[:, 2 * s + t, :], trs[(s, t)])

        def sub_lo(s, t):
            if sub_split and t == 1:
                h = sub_split
                nc.vector.tensor_tensor(wt_lo[:, 2 * s + t, :h],
                                        trs[(s, t)][:, :h],
                                        wt_hi[:, 2 * s + t, :h], sub)
                nc.gpsimd.tensor_tensor(wt_lo[:, 2 * s + t, h:],
                                        trs[(s, t)][:, h:],
                                        wt_hi[:, 2 * s + t, h:], sub)
                return
            eng = (nc.gpsimd if (t in sub_gps_ts
                                 or (t == 0 and s >= sub_gps_from))
                   else nc.vector)
            eng.tensor_tensor(wt_lo[:, 2 * s + t, :], trs[(s, t)],
                              wt_hi[:, 2 * s + t, :], sub)

        ten_first = {}  # m -> True if current tenancy is fresh (start leg)
        ten_last = {}   # m -> kk bound of current tenancy (exclusive)
        ten_skip = {}   # m -> skip group check (headless reload tenancy)

        def leg(m, lhsT, rhs, is_last):
            PE_LOG.append(("leg", m, nleg[m] // 3, nleg[m] % 3))
            nc.tensor.matmul(ps_of[m], lhsT, rhs,
                             start=ten_first.pop(m, False),
                             stop=is_last,
                             perf_mode=DR,
                             skip_group_check=ten_skip.get(m, False))
            nleg[m] += 1

        def legs_hi(m, s, is_last=False):
            rhs_hi = wt_hi[:, 2 * s:2 * s + 2, :]
            leg(m, xhi_sb[m][:, s], rhs_hi, False)
            leg(m, xlo_sb[m][:, s], rhs_hi, is_last)

        def legs_lo(m, s):
            # lo leg is always the tenancy's last emitted leg for kk s
            leg(m, xhi_sb[m][:, s], wt_lo[:, 2 * s:2 * s + 2, :],
                s == ten_last[m] - 1)

        def finalize(m):
            out_sb = outp.tile([P, O_SH], BF16, name="ob")
            part = part_sb.pop(m, None)
            feng = nc.gpsimd if fin_gps else nc.vector
            if part is not None:
                tmp = outp.tile([P, O_SH], F32, tag="tmpf", name="tf")
                feng.tensor_tensor(tmp, ps_of[m], bias_sb, add)
                feng.tensor_tensor(out_sb, tmp, part, add)
            else:
                feng.tensor_tensor(out_sb, ps_of[m], bias_sb, add)
            # the final block's DMA rides the idle Pool SWDGE queue, whose
            # issue latency is ~900ns shorter than SP's HWDGE path
            oq = (nc.gpsimd if (m == MB - 1 and out_q_last == "gps")
                  else nc.sync)
            oq.dma_start(out_t[m], out_sb)
            out_done.add(m)

        # ---- phase 1: stream dequant, wavefront of window m-blocks.
        # Per step s: PE runs hi-legs for kk=s-1, lo-legs for kk=s-2 (one
        # extra step of slack for the wt_lo residual), then transposes(s).
        # DVE/GPSIMD run this step's scales before last step's residual
        # subs; ACT casts trail the transposes.
        nhi = {}  # m -> next kk for hi legs
        nlo = {}  # m -> next kk for lo legs

        def emit_pe_legs(s):
            # interleave hi(kk=s-1) and lo(kk=s-2) legs per m
            for m in sorted(ps_of):
                lim = min(s - 1, cap[m],
                          nhi[m] + burst if s > rbase[m] else nhi[m])
                while nhi[m] < lim:
                    last_hi = (lo_defer and cap[m] < KK
                               and nhi[m] == cap[m] - 1)
                    legs_hi(m, nhi[m], last_hi)
                    nhi[m] += 1
                if lo_defer:
                    continue
                while nlo[m] < min(lim, s - lo_lag):
                    legs_lo(m, nlo[m])
                    nlo[m] += 1

        nx2 = 0
        lo_lag = 2 if sub_sched == "stag" else 1
        cap = {}      # m -> tenancy kk bound
        rbase = {}    # m -> catch-up ramp base step
        KH = spill_cap
        KH2 = spill_cap2 if spill_cap2 else spill_cap

        def open_ten(m, kk0, kk1, base_s, fresh):
            ps_of[m] = ps_mm.tile([P, O_SH], F32, name="acc")
            nhi[m] = nlo[m] = kk0
            if lo_defer and kk1 == KK:
                # deferred lo legs: the final tenancy emits all of them
                nlo[m] = 0
            cap[m] = ten_last[m] = kk1
            rbase[m] = base_s
            # every tenancy is a fresh accumulation group; spilled partials
            # are merged back at finalize (PSUM preload + headless
            # accumulation does not work on real HW)
            ten_first[m] = True
            ten_skip[m] = not fresh

        def flush_ten(m):
            # emit every remaining leg of the current tenancy before the
            # accumulator is read/spilled -- legs not yet emitted here
            # would otherwise be silently dropped
            while nhi[m] < cap[m]:
                last_hi = (lo_defer and cap[m] < KK
                           and nhi[m] == cap[m] - 1)
                legs_hi(m, nhi[m], last_hi)
                nhi[m] += 1
            if lo_defer and cap[m] < KK:
                return
            while nlo[m] < cap[m]:
                legs_lo(m, nlo[m])
                nlo[m] += 1

        def spill(m):
            part_sb[m] = prtp.tile([P, O_SH], BF16, name="part")
            nc.scalar.copy(part_sb[m], ps_of[m])
            del ps_of[m]

        for s in range(KK + 1 + lo_lag):
            if x2_queue == "gps" and s >= x2_start and nx2 < len(xq2):
                m = xq2[nx2]
                nc.gpsimd.dma_start(xhi_sb[m], xhi_d[m])
                nc.gpsimd.dma_start(xlo_sb[m], xlo_d[m])
                nx2 += 1
            if wlo_via != "pe" and sub_sched != "stag" and 1 <= s <= KK and s <= ramp_flip:
                # ramp: produce W_lo before the next scales so PE's lo-legs
                # are not starved while the pipeline fills
                for t in range(2):
                    sub_lo(s - 1, t)
            if s < KK:
                if preacc and s >= pa_from_s:
                    scale_preacc(s)
                elif s < s0_tsplit:
                    # pipeline-fill: emit all bit-planes as t-halves with t
                    # outer so the first transpose group's inputs finish
                    # ~2us earlier
                    for b in range(NB):
                        planes[(s, b)] = plp.tile([P, OT, 2, A], PDT,
                                                  name="plt")
                    for t in range(2):
                        for b in range(NB):
                            eng = (nc.gpsimd if b in scale_gps_bits
                                   else nc.vector)
                            eng.tensor_tensor(
                                planes[(s, b)][:, :, t],
                                bq_sb[s][:, :, b, t],
                                al_sb[:, :, 2 * s + t, b:b + 1]
                                .to_broadcast([P, OT, A]), mult)
                else:
                    for b in range(NB):
                        scale(s, b)
            if wlo_via != "pe" and sub_sched == "stag" and 1 <= s <= KK:
                nc.vector.tensor_tensor(
                    wt_lo[:, 2 * (s - 1), :], trs[(s - 1, 0)],
                    wt_hi[:, 2 * (s - 1), :], sub)
            if wlo_via == "pe":
                if 1 <= s <= KK:
                    for t in range(2):
                        neg_mm(s - 1, t)
                    for t in range(2):
                        cast_lo(s - 1, t)
            elif sub_sched == "stag":
                # staggered residuals: t0 on DVE one step stale (emitted
                # after the scales below), t1 on GPSIMD two steps stale so
                # neither engine waits on a fresh cast
                if 2 <= s <= KK + 1:
                    nc.gpsimd.tensor_tensor(
                        wt_lo[:, 2 * (s - 2) + 1, :], trs[(s - 2, 1)],
                        wt_hi[:, 2 * (s - 2) + 1, :], sub)
            elif 1 <= s <= KK and s > ramp_flip:
                for t in range(2):
                    sub_lo(s - 1, t)
            emit_pe_legs(s)
            if s < KK:
                if preacc and s >= pa_from_s:
                    transposes_preacc(s)
                else:
                    transposes(s)
                for t in range(2):
                    cast_hi(s, t)
                for m in admits[s]:
                    open_ten(m, 0, KH if spill_mode else KK, 0, True)
                if spill_mode and s == spill1:
                    for m in range(window):
                        flush_ten(m)
                        spill(m)
                    for m in range(window, 2 * window):
                        open_ten(m, 0, KH2, s, True)
                if spill_mode and s == reload1:
                    for m in range(window, 2 * window):
                        flush_ten(m)
                        spill(m)
                    for m in range(window):
                        open_ten(m, KH, KK, s, False)
        for m in sorted(ps_of):
            assert cap[m] == KK
            while nhi[m] < KK:
                legs_hi(m, nhi[m])
                nhi[m] += 1
            while nlo[m] < KK:
                legs_lo(m, nlo[m])
                nlo[m] += 1
            finalize(m)

        if x2_queue == "gps":
            while nx2 < len(xq2):
                m = xq2[nx2]
                nc.gpsimd.dma_start(xhi_sb[m], xhi_d[m])
                nc.gpsimd.dma_start(xlo_sb[m], xlo_d[m])
                nx2 += 1

        # ---- phase 2: remaining m-blocks
        if spill_mode:
            for m in range(window, 2 * window):
                open_ten(m, KH2, KK, 0, False)
                rbase.pop(m, None)
                while nhi[m] < KK:
                    legs_hi(m, nhi[m])
                    nhi[m] += 1
                while nlo[m] < KK:
                    legs_lo(m, nlo[m])
                    nlo[m] += 1
                finalize(m)
            rest = list(range(2 * window, MB))
        else:
            rest = list(range(window, MB))
        for m in rest[:-1] if tail_split else rest:
            open_ten(m, 0, KK, 0, True)
            rbase.pop(m, None)
            for s in range(KK):
                legs_hi(m, s)
                legs_lo(m, s)
            nhi[m] = nlo[m] = KK
            finalize(m)

        # every non-tail-split block must have emitted exactly 2*KK hi legs
        # and KK lo legs -- guards against silently dropped legs
        for m in range(MB - 1 if tail_split else MB):
            assert nleg[m] == 3 * KK, (m, nleg[m])
        if tail_split:
            # last m-block in o-halves: half A finalizes and DMAs while
            # half B's legs still run, shortening the drain tail. Each half
            # gets its own PSUM tile (from the long-idle transpose pool) so
            # half B's legs don't serialize behind half A's finalize read.
            m = MB - 1
            nleg[m] = 0
            HO = O_SH // 2
            for h in range(2):
                osl = slice(h * HO, (h + 1) * HO)
                psh = ps_tr.tile([P, OT * P], F32, name="acch", tag="trt")
                first = True
                for s in range(KK):
                    rhs_hi = wt_hi[:, 2 * s:2 * s + 2, osl]
                    rhs_lo = wt_lo[:, 2 * s:2 * s + 2, osl]
                    lh = xhi_sb[m][:, s]
                    ll = xlo_sb[m][:, s]
                    for lhsT, rhs in ((lh, rhs_hi), (ll, rhs_hi),
                                      (lh, rhs_lo)):
                        PE_LOG.append(("tleg", m, h, s))
                        nc.tensor.matmul(
                            psh[:, :HO], lhsT, rhs,
                            start=first, stop=(s == KK - 1 and rhs is rhs_lo),
                            perf_mode=DR, skip_group_check=True)
                        first = False
                ob = outp.tile([P, HO], BF16, tag="obh", name="obh")
                nc.vector.tensor_tensor(ob, psh[:, :HO],
                                        bias_sb[:, osl], add)
                nc.sync.dma_start(out_t[m][:, osl], ob)
            out_done.add(m)

    _legalize_waits(nc)
    return nc


def _stage_inputs(input, binary, alpha, bias, xlo_dt="e4", head_merge=False,
                  act_on=False):
    np_e4 = mybir.dt.np(E4)
    np_xlo = mybir.dt.np({"e5": E5, "e4": E4}[xlo_dt])

    x = np.ascontiguousarray(np.asarray(input, dtype=np.float32)).reshape(MS, I)
    x_hi = x.astype(np_e4)
    x_lo = (x - x_hi.astype(np.float32)).astype(np_xlo)
    # [MS, I] -> [m, j, kk, t, p] -> [m, p, kk, t, j]
    def relayout_x(a):
        return np.ascontiguousarray(
            a.reshape(MB, P, KK, 2, P).transpose(0, 4, 2, 3, 1))
    xhi = relayout_x(x_hi)
    xlo = relayout_x(x_lo)

    binary = np.asarray(binary, dtype=np.float32)
    alpha = np.ascontiguousarray(np.asarray(alpha, dtype=np.float32))
    bias = np.asarray(bias, dtype=np.float32)

    in_maps = []
    for c in range(N_CORES):
        sl = slice(c * O_SH, (c + 1) * O_SH)
        # binary [512, G, A, NB] -> [ot, p, kk, t, a, b] -> [kk, p, ot, b, t, a]
        bc = binary[sl].reshape(OT, P, KK, 2, A, NB)
        bq = np.ascontiguousarray(bc.transpose(2, 1, 0, 5, 3, 4)).astype(np_e4)
        al = np.ascontiguousarray(
            alpha[sl].reshape(OT, P, G, NB).transpose(1, 0, 2, 3))
        im = {
            "xhi": xhi,
            "xlo": xlo,
            "bq": bq,
            "biasr": np.ascontiguousarray(
                np.broadcast_to(bias[sl][None, :], (P, O_SH))),
        }
        if head_merge:
            al16 = al.astype(np.float16)
            im["head"] = np.ascontiguousarray(np.concatenate(
                [al16.reshape(P, -1).view(np.uint8),
                 bq[0].reshape(P, -1).view(np.uint8)], axis=1))
            if act_on:
                im["al"] = al
        else:
            im["al"] = al
        in_maps.append(im)
    return in_maps


# best-known schedule (TimelineSim 113008 ns vs 125626 for the old default):
# t-split first scales; all bq chunks on the SP DMA queue ahead of the x
# stream; fp16-alpha+bq0 merged into a single head DMA; one extra block
# admitted at s=0; last block split in o-halves to overlap the drain tail;
# steps >= 10 pre-accumulate the 3 bit-planes on DVE (single-plane PE
# transposes), viable there because post-spill leg backlog keeps PE fed;
# ACT takes one t-half of the b1 scale muls in steps 5-9 (reading a
# separate f32 alpha -- walrus' lower_act rejects fp16 scale APs) to
# absorb the DVE drift. GPSIMD cannot read PSUM (hard walrus constraint),
# so the residual subs must stay on DVE.
DEFAULT_CFG = dict(s0_tsplit=2, bq_q="sp", bq_head=3, xw_split=True,
                   admit_off=3, head_merge=True, tail_split=True,
                   preacc=2, plane_dt="fp16", pa_tmp_bufs=2, x_bufs=14,
                   pa_from_s=10, act_from=5, act_until=10, act_b=1)


def kernel(input, binary, alpha, bias, _trace=False, **cfg):
    full = dict(DEFAULT_CFG)
    full.update(cfg)
    cfg = full
    key = tuple(sorted(cfg.items()))
    if key not in _CACHED:
        _CACHED[key] = build_nc(**cfg)
    nc = _CACHED[key]
    in_maps = _stage_inputs(input, binary, alpha, bias,
                            xlo_dt=cfg.get("xlo_dt", "e4"),
                            head_merge=cfg.get("head_merge", False),
                            act_on=cfg.get("act_from", 99) < 99)
    res = run_bass_kernel_spmd(nc, in_maps, core_ids=list(range(N_CORES)),
                               trace=_trace)
    outs = []
    for c in range(N_CORES):
        o = np.asarray(res.results[c]["out"]).astype(np.float32)
        if cfg.get("ps_direct", False):
            o[(MB - 1) * P:, :] = np.asarray(res.results[c]["o15"])
        outs.append(o)
    out = np.concatenate(outs, axis=1).reshape(B, S, O)
    kernel.last_result = res
    return out

